# revision 23
# baseline (speedup 1.0000x reference)
"""AgentAttention Trainium2 kernel — 8-core batch-parallel (2 batches/core).

Decomposition (validated in mirror2.py against the reference):
  - host: x transposed to ch-major (bf16 + fp8 copies); position biases
    precomputed as exp(bias) factors; v_b / dwc_b / proj_b folded into a
    host-side additive correction (softmax rows sum to 1); k_b dropped
    (softmax shift-invariant).
  - device (per batch): agent tokens pooled directly from x on GpSimd
    (pooling is linear) and projected once (49 tokens); per-batch composed
    score weights W1h = k_w @ agent_s^T and W2h = q_w @ agent_s^T built on
    the PE and quantized to fp8, so both attention score matrices come
    straight from x via fp8 DoubleRow matmuls without materializing Q or K.
    Stage-1 V is also fp8 DoubleRow (only feeds softmax averaging); the
    dwc/direct-path V stays bf16.  Stage-2 weights are normalized *before*
    the output matmul (denominators accumulate across head pairs into one
    [8,448] PSUM bank, fast-reciprocal once, K=8 broadcast matmul), which
    lets the depthwise 3x3 conv taps accumulate directly into the stage-2
    output PSUM bank.  C-phase is software-pipelined (produce c / consume
    c-1) to keep the PE dense across the reciprocal dependency.
"""
import numpy as np
import ml_dtypes

BF = ml_dtypes.bfloat16
F8 = ml_dtypes.float8_e4m3
NCORES = 8
B = 2              # batches per core
N = 3136
H = W = 56
CT = 4             # 128-channel tiles
HP = 4             # head pairs
A = 49
C7 = 448           # 8 image rows
VR = 58            # padded image row width
VF = VR * VR + 12  # flat padded V image + alignment slack
CH = [(i * 128, min(128, N - i * 128)) for i in range(25)]
W8 = 16384.0       # fp8 scale for composed score weights
X8 = 32.0          # fp8 scale for x
V8 = 2048.0        # fp8 scale for v weights

_CACHE = {}


def _lin_weights(in_size, out_size):
    scale = in_size / out_size
    src = (np.arange(out_size, dtype=np.float32) + 0.5) * scale - 0.5
    src = np.maximum(src, 0.0)
    i0 = np.minimum(np.floor(src).astype(np.int32), in_size - 1)
    i1 = np.minimum(i0 + 1, in_size - 1)
    w = (src - i0.astype(np.float32)).astype(np.float32)
    return i0, i1, w


def _resize_matrix(in_size, out_size):
    i0, i1, w = _lin_weights(in_size, out_size)
    M = np.zeros((out_size, in_size), np.float32)
    M[np.arange(out_size), i0] += 1.0 - w
    M[np.arange(out_size), i1] += w
    return M


def _build_nc():
    from contextlib import ExitStack
    import concourse.bacc as bacc
    import concourse.tile as tile
    from concourse import mybir

    fp32 = mybir.dt.float32
    bf16 = mybir.dt.bfloat16
    fp8 = mybir.dt.float8e4
    AF = mybir.ActivationFunctionType
    AX = mybir.AxisListType
    DR = mybir.MatmulPerfMode.DoubleRow

    nc = bacc.Bacc("TRN2", target_bir_lowering=False)
    xT_d = nc.dram_tensor("xT", (128, B, CT, N), bf16, kind="ExternalInput")
    x8_d = nc.dram_tensor("x8", (128, B, CT, N), fp8, kind="ExternalInput")
    wq_d = nc.dram_tensor("wq", (128, CT, 512), bf16, kind="ExternalInput")
    qsb_d = nc.dram_tensor("qsb", (128, CT), fp32, kind="ExternalInput")
    kwT_d = nc.dram_tensor("kwT", (128, HP, CT, 128), bf16, kind="ExternalInput")
    qwT_d = nc.dram_tensor("qwT", (128, HP, CT, 128), bf16, kind="ExternalInput")
    qbT_d = nc.dram_tensor("qbT", (128, HP), bf16, kind="ExternalInput")
    wv_d = nc.dram_tensor("wv", (128, CT, 512), bf16, kind="ExternalInput")
    wv8_d = nc.dram_tensor("wv8", (128, CT, 512), fp8, kind="ExternalInput")
    pw_d = nc.dram_tensor("pw", (128, CT, 512), bf16, kind="ExternalInput")
    wdiag_d = nc.dram_tensor("wdiag", (128, 36, 128), bf16, kind="ExternalInput")
    wsc_d = nc.dram_tensor("wsc", (128, CT, 9), fp32, kind="ExternalInput")
    eb1_d = nc.dram_tensor("eb1", (128, 25, HP, 128), bf16, kind="ExternalInput")
    eb2_d = nc.dram_tensor("eb2", (128, HP, 7, C7), bf16, kind="ExternalInput")
    ones8_d = nc.dram_tensor("ones8", (128, HP, 8), bf16, kind="ExternalInput")
    sel2_d = nc.dram_tensor("sel2", (8, HP, 128), bf16, kind="ExternalInput")
    out_d = nc.dram_tensor("out", (B, N, 512), fp32, kind="ExternalOutput")

    with ExitStack() as ctx:
        tc = ctx.enter_context(tile.TileContext(nc))
        consts = ctx.enter_context(tc.tile_pool(name="consts", bufs=1))
        xtp = ctx.enter_context(tc.tile_pool(name="xtp", bufs=1))
        x8p = ctx.enter_context(tc.tile_pool(name="x8p", bufs=2))
        usp = ctx.enter_context(tc.tile_pool(name="usp", bufs=2))
        vpadp = ctx.enter_context(tc.tile_pool(name="vpadp", bufs=1))
        batch1 = ctx.enter_context(tc.tile_pool(name="batch1", bufs=1))
        work = ctx.enter_context(tc.tile_pool(name="work", bufs=3))
        workB = ctx.enter_context(tc.tile_pool(name="workB", bufs=2))
        otp = ctx.enter_context(tc.tile_pool(name="otp", bufs=2))
        perb = ctx.enter_context(tc.tile_pool(name="perb", bufs=3))
        e2bp = ctx.enter_context(tc.tile_pool(name="e2bp", bufs=8))
        etnp = ctx.enter_context(tc.tile_pool(name="etnp", bufs=4))
        accp = ctx.enter_context(tc.tile_pool(name="accp", bufs=2))
        biasp = ctx.enter_context(tc.tile_pool(name="biasp", bufs=2))
        ps_mm = ctx.enter_context(tc.tile_pool(name="psmm", bufs=6, space="PSUM"))
        ps_avp = ctx.enter_context(tc.tile_pool(name="psavp", bufs=1, space="PSUM"))

        wq_s = consts.tile([128, CT, 512], bf16)
        nc.sync.dma_start(out=wq_s, in_=wq_d[:, :, :])
        qsb_s = consts.tile([128, CT], fp32)
        nc.sync.dma_start(out=qsb_s, in_=qsb_d[:, :])
        kwT_s = consts.tile([128, HP, CT, 128], bf16)
        nc.sync.dma_start(out=kwT_s, in_=kwT_d[:, :, :, :])
        qwT_s = consts.tile([128, HP, CT, 128], bf16)
        nc.sync.dma_start(out=qwT_s, in_=qwT_d[:, :, :, :])
        qbT_s = consts.tile([128, HP], bf16)
        nc.sync.dma_start(out=qbT_s, in_=qbT_d[:, :])
        wv_s = consts.tile([128, CT, 512], bf16)
        nc.sync.dma_start(out=wv_s, in_=wv_d[:, :, :])
        wv8_s = consts.tile([128, CT, 512], fp8)
        nc.sync.dma_start(out=wv8_s, in_=wv8_d[:, :, :])
        pw_s = consts.tile([128, CT, 512], bf16)
        nc.sync.dma_start(out=pw_s, in_=pw_d[:, :, :])
        wdiag_s = consts.tile([128, 36, 128], bf16)
        nc.sync.dma_start(out=wdiag_s, in_=wdiag_d[:, :, :])
        wsc_s = consts.tile([128, CT, 9], fp32)
        nc.sync.dma_start(out=wsc_s, in_=wsc_d[:, :, :])
        ones8_s = consts.tile([128, HP, 8], bf16)
        nc.sync.dma_start(out=ones8_s, in_=ones8_d[:, :, :])
        sel2_s = consts.tile([8, HP, 128], bf16)
        nc.sync.dma_start(out=sel2_s, in_=sel2_d[:, :, :])
        zt = consts.tile([1, 512], bf16)
        nc.vector.memset(zt, 0.0)

        # persistent zero-padded ch-major V image (pad stays zero across
        # batches; interior rewritten per batch)
        vpad = vpadp.tile([128, CT, VF], bf16)
        vimg = vpad[:, :, 0:VR * VR].rearrange("p c (y x) -> p c y x", y=VR)
        nc.vector.memset(vimg[:, :, 0, :], 0.0)
        nc.vector.memset(vimg[:, :, 57, :], 0.0)
        nc.vector.memset(vimg[:, :, 1:57, 0:1], 0.0)
        nc.vector.memset(vimg[:, :, 1:57, 57:58], 0.0)
        nc.vector.memset(vpad[:, :, VR * VR:VF], 0.0)

        def a_dma(b, S):
            xT = xtp.tile([128, CT, N], bf16, tag="xt")
            x8 = x8p.tile([128, CT, N], fp8, tag="x8")
            for kt in range(CT):
                nc.gpsimd.dma_start(out=xT[:, kt, :], in_=xT_d[:, b, kt, :])
            for kt in range(CT):
                nc.gpsimd.dma_start(out=x8[:, kt, :], in_=x8_d[:, b, kt, :])
            S.update(xT=xT, x8=x8)

        def a_pool(b, S):
            # pool x over 8x8 windows -> px [128, CT, 49] sums (DVE; emitted
            # where DVE is otherwise idle)
            xT = S['xT']
            px = batch1.tile([128, CT, A], bf16, tag="px")
            for ct in range(CT):
                p1 = workB.tile([128, 392], fp32, tag="pool1")
                p1v = p1.rearrange("p (b y q) -> p b y q", b=8, y=7)
                xv = xT[:, ct, :].rearrange("p (b y q r) -> p b y q r", b=8, y=7, q=7)
                for yb in range(8):
                    nc.vector.reduce_sum(out=p1v[:, yb, :, :], in_=xv[:, yb, :, :, :],
                                         axis=AX.X)
                with nc.allow_low_precision(reason="bf16 agent pooling"):
                    nc.vector.reduce_sum(
                        out=px[:, ct, :].rearrange("p (a c) -> p a c", a=7),
                        in_=p1.rearrange("p (yq yr xq) -> p yq xq yr", yq=7, yr=8),
                        axis=AX.X,
                    )
            S.update(px=px)

        def emit_vt(xT, ct, r):
            ps = ps_mm.tile([128, 512], fp32, tag="mm")
            for kt in range(CT):
                nc.tensor.matmul(
                    ps[:, 0:C7],
                    wv_s[:, kt, ct * 128:(ct + 1) * 128],
                    xT[:, kt, r * C7:(r + 1) * C7],
                    start=(kt == 0), stop=(kt == 3),
                )
            nc.vector.tensor_copy(
                out=vimg[:, ct, 8 * r + 1:8 * r + 9, 1:57],
                in_=ps[:, 0:C7].rearrange("p (y x) -> p y x", y=8))

        def a_comp(b, S):
            px = S['px']
            for r_ in range(2):
                for ct_ in range(CT):
                    emit_vt(S['xT'], ct_, r_)
            # scaled agent tokens, block-diagonal per head pair
            bd = batch1.tile([128, CT, 128], bf16, tag="bd")
            nc.vector.memset(bd, 0.0)
            for ct in range(CT):
                psA = ps_mm.tile([128, 512], fp32, tag="mm")
                for kt in range(CT):
                    nc.tensor.matmul(
                        psA[:, 0:A],
                        wq_s[:, kt, ct * 128:(ct + 1) * 128],
                        px[:, kt, :],
                        start=(kt == 0), stop=(kt == 3),
                    )
                for e in range(2):
                    nc.scalar.activation(
                        out=bd[64 * e:64 * e + 64, ct, 64 * e:64 * e + A],
                        in_=psA[64 * e:64 * e + 64, 0:A],
                        func=AF.Identity, bias=qsb_s[64 * e:64 * e + 64, ct:ct + 1],
                        scale=1.0 / 64.0,
                    )
            qb_col = batch1.tile([128, HP], fp32, tag="qbcol")
            for hp in range(HP):
                psQ = ps_mm.tile([128, 512], fp32, tag="mm")
                nc.tensor.matmul(psQ[:, 0:1], bd[:, hp, :], qbT_s[:, hp:hp + 1],
                                 start=True, stop=True)
                nc.scalar.copy(out=qb_col[:, hp:hp + 1], in_=psQ[:, 0:1])
            # composed fp8 score weights (x W8 for fp8 resolution)
            W1s = batch1.tile([128, CT, 512], fp8, tag="W1s")
            W2s = batch1.tile([128, CT, 512], fp8, tag="W2s")
            for kt in range(CT):
                psW1 = ps_mm.tile([128, 512], fp32, tag="mm")
                for hp in range(HP):
                    nc.tensor.matmul(
                        psW1[:, hp * 128:hp * 128 + 128],
                        kwT_s[:, hp, kt, :], bd[:, hp, :],
                        start=True, stop=True,
                    )
                with nc.allow_low_precision(reason="fp8 composed score weights"):
                    nc.scalar.activation(out=W1s[:, kt, :], in_=psW1,
                                         func=AF.Identity, scale=W8)
                psW2 = ps_mm.tile([128, 512], fp32, tag="mm")
                for hp in range(HP):
                    nc.tensor.matmul(
                        psW2[:, hp * 128:hp * 128 + 128],
                        qwT_s[:, hp, kt, :], bd[:, hp, :],
                        start=True, stop=True,
                    )
                with nc.allow_low_precision(reason="fp8 composed score weights"):
                    nc.scalar.activation(out=W2s[:, kt, :], in_=psW2,
                                         func=AF.Identity, scale=W8)
            S.update(qb_col=qb_col, W1s=W1s, W2s=W2s)

        def phase_b(b, S):
            xT, x8, W1s = S['xT'], S['x8'], S['W1s']
            vt_units = [(ct_, r_) for r_ in range(2, 7) for ct_ in range(CT)]

            avpAB = ps_avp.tile([128, 512], fp32, tag="avpA")
            avpCD = ps_avp.tile([128, 512], fp32, tag="avpB")
            avps = [avpAB[:, 0:130], avpAB[:, 256:386],
                    avpCD[:, 0:130], avpCD[:, 256:386]]
            # claim + zero both regions in one accumulation group per bank so
            # the per-hp accumulations can all run start=False (two groups
            # sharing a bank with their own start=True corrupts each other)
            nc.tensor.matmul(avpAB[:, 0:386], zt[0:1, 0:128], zt[0:1, 0:386],
                             start=True, stop=True)
            nc.tensor.matmul(avpCD[:, 0:386], zt[0:1, 0:128], zt[0:1, 0:386],
                             start=True, stop=True)
            prev = None

            def emit_avp(pi, pet4, pv65, pcs):
                for hp in range(HP):
                    nc.tensor.matmul(
                        avps[hp],
                        pet4[0:pcs, hp, :],
                        pv65[0:pcs, 2 * hp:2 * hp + 2, :],
                        start=False, stop=(pi == 24), skip_group_check=True,
                    )

            for ci, (t0, cs) in enumerate(CH):
                v65 = perb.tile([128, 8, 65], bf16, tag="v65")
                ps = ps_mm.tile([128, 512], fp32, tag="mm")
                for i2 in range(2):
                    nc.tensor.matmul(
                        ps[0:cs, :], x8[:, 2 * i2:2 * i2 + 2, t0:t0 + cs],
                        wv8_s[:, 2 * i2:2 * i2 + 2, :],
                        start=(i2 == 0), stop=(i2 == 1), perf_mode=DR,
                    )
                nc.scalar.activation(
                    out=v65[0:cs, :, 0:64],
                    in_=ps[0:cs, :].rearrange("p (h d) -> p h d", h=8),
                    func=AF.Identity, scale=1.0 / (X8 * V8))
                nc.vector.memset(v65[0:cs, :, 64:65], 1.0)
                e1b = biasp.tile([128, HP, 128], bf16, tag="eb1")
                nc.gpsimd.dma_start(out=e1b, in_=eb1_d[:, ci, :, :])
                ps1 = ps_mm.tile([128, 512], fp32, tag="mm")
                for i2 in range(2):
                    nc.tensor.matmul(
                        ps1[0:cs, :], x8[:, 2 * i2:2 * i2 + 2, t0:t0 + cs],
                        W1s[:, 2 * i2:2 * i2 + 2, :],
                        start=(i2 == 0), stop=(i2 == 1), perf_mode=DR,
                    )
                et4 = work.tile([128, HP, 128], bf16, tag="e1")
                nc.scalar.activation(
                    out=et4[0:cs, :, :].rearrange("p h a -> p (h a)"),
                    in_=ps1[0:cs, 0:512], func=AF.Exp, scale=1.0 / (X8 * W8))
                nc.vector.tensor_mul(out=et4[0:cs, :, :], in0=et4[0:cs, :, :], in1=e1b[0:cs, :, :])
                if vt_units:
                    emit_vt(xT, *vt_units.pop(0))
                if prev is not None:
                    emit_avp(*prev)
                prev = (ci, et4, v65, cs)
            for u_ in list(vt_units):
                emit_vt(xT, *u_)
            emit_avp(*prev)
            avbds = []
            for hp in range(HP):
                avbd = batch1.tile([128, 128], bf16, tag=f"avbd{hp}")
                nc.vector.memset(avbd, 0.0)
                rr = workB.tile([128, 1], fp32, tag="rr")
                for e in range(2):
                    nc.vector.reciprocal(out=rr[64 * e:64 * e + A, :],
                                         in_=avps[hp][64 * e:64 * e + A, 65 * e + 64:65 * e + 65])
                    nc.vector.tensor_scalar_mul(
                        out=avbd[64 * e:64 * e + A, 64 * e:64 * e + 64],
                        in0=avps[hp][64 * e:64 * e + A, 65 * e:65 * e + 64],
                        scalar1=rr[64 * e:64 * e + A, :],
                    )
                avbds.append(avbd)
            S.update(avbds=avbds)

        def phase_c(b, S):
            x8, W2s, qb_col, avbds = S['x8'], S['W2s'], S['qb_col'], S['avbds']
            u_s = usp.tile([128, CT, N], bf16, tag="us")
            prev = None

            def consume(pc, pets, prec):
                sl = slice(pc * C7, (pc + 1) * C7)
                etns = []
                for hp in range(HP):
                    psB = ps_mm.tile([128, 512], fp32, tag="mm")
                    nc.tensor.matmul(psB[:, 0:C7], sel2_s[:, hp, :], prec,
                                     start=True, stop=True)
                    etn = etnp.tile([128, C7], bf16, tag="etn")
                    nc.vector.tensor_mul(out=etn, in0=pets[hp], in1=psB[:, 0:C7])
                    etns.append(etn)
                for hp in range(HP):
                    psU = ps_mm.tile([128, 512], fp32, tag="mm")
                    nc.tensor.matmul(psU[:, 0:C7], avbds[hp], etns[hp],
                                     start=True, stop=False)
                    for j in range(9):
                        dy, dx = j // 3, j % 3
                        nc.tensor.matmul(
                            psU[:, 0:C7].rearrange("p (y x) -> p y x", y=8),
                            wdiag_s[:, hp * 9 + j, :],
                            vimg[:, hp, 8 * pc + dy:8 * pc + dy + 8, dx:dx + 56],
                            start=False, stop=(j == 8),
                        )
                    nc.scalar.copy(out=u_s[:, hp, sl], in_=psU[:, 0:C7])

            for c in range(7):
                sl = slice(c * C7, (c + 1) * C7)
                ets = []
                for hp in range(HP):
                    e2b = biasp.tile([128, C7], bf16, tag="eb2")
                    nc.sync.dma_start(out=e2b, in_=eb2_d[:, hp, c, :])
                    ps2 = ps_mm.tile([128, 512], fp32, tag="mm")
                    for i2 in range(2):
                        nc.tensor.matmul(
                            ps2[0:128, 0:C7],
                            W2s[:, 2 * i2:2 * i2 + 2, hp * 128:hp * 128 + 128],
                            x8[:, 2 * i2:2 * i2 + 2, sl],
                            start=(i2 == 0), stop=(i2 == 1), perf_mode=DR,
                        )
                    et2 = work.tile([128, C7], bf16, tag="e2")
                    nc.scalar.activation(out=et2, in_=ps2[0:128, 0:C7], func=AF.Exp,
                                         bias=qb_col[:, hp:hp + 1], scale=1.0 / (X8 * W8))
                    et2b = e2bp.tile([128, C7], bf16, tag="e2b")
                    nc.vector.tensor_mul(out=et2b, in0=et2, in1=e2b)
                    ets.append(et2b)
                if prev is not None:
                    consume(*prev)
                psD8 = ps_mm.tile([8, 512], fp32, tag="mm")
                for hp in range(HP):
                    nc.tensor.matmul(psD8[:, 0:C7], ones8_s[:, hp, :], ets[hp],
                                     start=(hp == 0), stop=(hp == 3))
                den8 = workB.tile([8, C7], fp32, tag="den8")
                nc.scalar.copy(out=den8, in_=psD8[:, 0:C7])
                rec32 = workB.tile([8, C7], fp32, tag="rec32")
                nc.vector.reciprocal_approx_fast(out=rec32, in_=den8)
                rec8 = workB.tile([8, C7], bf16, tag="rec8")
                with nc.allow_low_precision(reason="bf16 1/den for broadcast"):
                    nc.vector.tensor_copy(out=rec8, in_=rec32)
                prev = (c, ets, rec8)
            consume(*prev)
            S.update(u_s=u_s)

        def phase_p(b, S):
            u_s = S['u_s']
            for ci, (t0, cs) in enumerate(CH):
                psP = ps_mm.tile([128, 512], fp32, tag="mm")
                for kt in range(CT):
                    nc.tensor.matmul(
                        psP[0:cs, :], u_s[:, kt, t0:t0 + cs], pw_s[:, kt, :],
                        start=(kt == 0), stop=(kt == 3),
                    )
                ot = otp.tile([128, 512], fp32, tag="ot")
                nc.vector.tensor_copy(out=ot[0:cs, :], in_=psP[0:cs, :])
                nc.sync.dma_start(out=out_d[b, t0:t0 + cs, :], in_=ot[0:cs, :])

        S0, S1 = {}, {}
        a_dma(0, S0)
        a_pool(0, S0)
        a_comp(0, S0)
        phase_b(0, S0)
        a_dma(1, S1)
        phase_c(0, S0)
        a_pool(1, S1)
        phase_p(0, S0)
        a_comp(1, S1)
        phase_b(1, S1)
        phase_c(1, S1)
        phase_p(1, S1)
    return nc


def _host_prep(q_w, q_b, kv_w, kv_b, proj_w, proj_b, dwc_w, dwc_b,
               an_bias, na_bias, ah_bias, aw_bias, ha_bias, wa_bias):
    heads, dh = 8, 64
    scale = dh ** -0.5
    q_w = np.asarray(q_w, np.float32); q_b = np.asarray(q_b, np.float32)
    kv_w = np.asarray(kv_w, np.float32); kv_b = np.asarray(kv_b, np.float32)
    proj_w = np.asarray(proj_w, np.float32); proj_b = np.asarray(proj_b, np.float32)
    dwc_w = np.asarray(dwc_w, np.float32); dwc_b = np.asarray(dwc_b, np.float32)

    Rh = _resize_matrix(7, H)
    Rw = _resize_matrix(7, W)
    an = np.asarray(an_bias, np.float32); na = np.asarray(na_bias, np.float32)
    pb1 = np.einsum('yi,haij,xj->hayx', Rh, an, Rw).reshape(heads, A, N)
    pb2 = (np.asarray(ah_bias, np.float32) + np.asarray(aw_bias, np.float32)).reshape(heads, A, N)
    bias1 = pb1 + pb2                                      # (h, a, n)
    ab1 = np.einsum('yi,haij,xj->hayx', Rh, na, Rw).reshape(heads, A, N)
    ab2 = (np.asarray(ha_bias, np.float32) + np.asarray(wa_bias, np.float32)).reshape(heads, N, A)
    bias2 = ab1.transpose(0, 2, 1) + ab2                   # (h, n, a)

    k_w = kv_w[:, :512]
    v_w = kv_w[:, 512:]
    v_b = kv_b[512:]
    dwc9 = dwc_w.reshape(512, 9)

    wq_t = np.ascontiguousarray(
        (q_w * scale).reshape(4, 128, 512).transpose(1, 0, 2)).astype(BF)
    qsb_t = np.ascontiguousarray((q_b * scale).reshape(4, 128).T).astype(np.float32)
    kwT_t = np.ascontiguousarray(
        k_w.reshape(CT, 128, HP, 128).transpose(3, 2, 0, 1)).astype(BF)
    qwT_t = np.ascontiguousarray(
        q_w.reshape(CT, 128, HP, 128).transpose(3, 2, 0, 1)).astype(BF)
    qbT_t = np.ascontiguousarray(q_b.reshape(4, 128).T).astype(BF)
    wv_t = np.ascontiguousarray(v_w.reshape(4, 128, 512).transpose(1, 0, 2)).astype(BF)
    wv8_t = (np.asarray(wv_t, np.float32) * V8).astype(F8)
    pw_t = np.ascontiguousarray(proj_w.reshape(4, 128, 512).transpose(1, 0, 2)).astype(BF)
    wdiag_t = np.zeros((128, 36, 128), np.float32)
    for ct_ in range(4):
        for j_ in range(9):
            wdiag_t[np.arange(128), ct_ * 9 + j_, np.arange(128)] = dwc9[ct_ * 128 + np.arange(128), j_]
    wdiag_t = wdiag_t.astype(BF)

    # eb1 (128, 25, HP, 128): [p, ch, hp, 64e+a] = exp(bias1)[2hp+e, a, 128ch+p]
    e1 = np.exp(bias1)                                     # (h, a, n)
    e1p = np.ones((128, 25, HP, 128), np.float32)
    e1t = e1.transpose(2, 0, 1)                            # (n, h, a)
    for ci, (t0, cs) in enumerate(CH):
        blk = e1t[t0:t0 + cs]                              # (cs, h, a)
        for hp_ in range(HP):
            e1p[:cs, ci, hp_, 0:49] = blk[:, 2 * hp_, :]
            e1p[:cs, ci, hp_, 64:113] = blk[:, 2 * hp_ + 1, :]
    eb1_t = e1p.astype(BF)

    # eb2 (128, HP, 7, 448): [64e+a, hp, c, t'] = exp(bias2)[2hp+e, 448c+t', a]
    e2 = np.exp(bias2)                                     # (h, n, a)
    e2p = np.zeros((128, HP, 7, C7), np.float32)
    for hp_ in range(HP):
        for e in range(2):
            e2p[64 * e:64 * e + 49, hp_] = e2[2 * hp_ + e].reshape(7, C7, A).transpose(2, 0, 1)
    eb2_t = e2p.astype(BF)

    # ones8 (128, HP, 8): [64e+a, hp, 2hp+e] = 1 for a < 49
    ones8_t = np.zeros((128, HP, 8), np.float32)
    for hp_ in range(HP):
        for e in range(2):
            ones8_t[64 * e:64 * e + 49, hp_, 2 * hp_ + e] = 1.0
    ones8_t = ones8_t.astype(BF)

    # sel2 (8, HP, 128): [2hp+e, hp, 64e+p'] = 1
    sel2_t = np.zeros((8, HP, 128), np.float32)
    for hp_ in range(HP):
        for e in range(2):
            sel2_t[2 * hp_ + e, hp_, 64 * e:64 * e + 64] = 1.0
    sel2_t = sel2_t.astype(BF)

    # host additive correction (v_b + dwc_b + proj_b, exact via softmax-sum-1)
    Mv = np.zeros((9, H, W), np.float32)
    for j in range(9):
        dy, dx = j // 3 - 1, j % 3 - 1
        Mv[j, max(0, -dy):H - max(0, dy), max(0, -dx):W - max(0, dx)] = 1.0
    S = np.einsum('jt,cj->tc', Mv.reshape(9, N), dwc9)
    corr = v_b[None, :] * (1.0 + S) + dwc_b[None, :]
    corr_out = (corr @ proj_w + proj_b[None, :]).astype(np.float32)   # (n, 512)

    wsc_t = np.ascontiguousarray(dwc9.reshape(CT, 128, 9).transpose(1, 0, 2)).astype(np.float32)

    return dict(wq=wq_t, qsb=qsb_t, kwT=kwT_t, qwT=qwT_t, qbT=qbT_t,
                wv=wv_t, wv8=wv8_t, pw=pw_t, wdiag=wdiag_t, wsc=wsc_t,
                eb1=eb1_t, eb2=eb2_t, ones8=ones8_t, sel2=sel2_t), corr_out


def kernel(**inputs):
    from concourse.bass_utils import run_bass_kernel_spmd

    x = np.asarray(inputs['x'], np.float32)                # (16, 3136, 512)
    shared, corr_out = _host_prep(
        inputs['q_w'], inputs['q_b'], inputs['kv_w'], inputs['kv_b'],
        inputs['proj_w'], inputs['proj_b'], inputs['dwc_w'], inputs['dwc_b'],
        inputs['an_bias'], inputs['na_bias'], inputs['ah_bias'],
        inputs['aw_bias'], inputs['ha_bias'], inputs['wa_bias'])

    # xT per core: (128, B, CT, N) bf16/fp8 ; [p, b, kt, t] = x[2c+b, t, 128kt+p]
    xr = np.ascontiguousarray(
        x.reshape(NCORES, B, N, CT, 128).transpose(0, 4, 1, 3, 2))
    xb = xr.astype(BF)
    x8 = (xr * X8).astype(F8)

    if 'nc' not in _CACHE:
        nc = _build_nc()
        nc.finalize()
        _CACHE['nc'] = nc
    nc = _CACHE['nc']

    in_maps = []
    for c in range(NCORES):
        m = {'xT': xb[c], 'x8': x8[c]}
        m.update(shared)
        in_maps.append(m)
    res = run_bass_kernel_spmd(nc, in_maps, core_ids=list(range(NCORES)))
    _CACHE['last_res'] = res
    outs = res.results
    full = np.concatenate([np.asarray(o['out']).reshape(B, N, 512) for o in outs], axis=0)
    full = full + corr_out[None, :, :]
    return full.astype(np.float32)
